# revision 42
# baseline (speedup 1.0000x reference)
"""Causal self-attention + depthwise-conv + out-proj fused TRN2 kernel (v3).

Model (B=4, T=2048, C=1024, H=16, D=64, conv K=4):
    qkv = x @ W_qkv.T ; causal softmax attention per head ;
    y2 = attn + causal_depthwise_conv(attn) + conv_b ; out = y2 @ W_out.T

Sharding over 8 NeuronCores: core c -> (batch b = c//2, head-group g = c%2).
Each core computes q/k/v for its 8 heads, bf16 flash-style causal attention
in transposed [d, t] layout (exp softmax without max subtraction), the
depthwise conv as diagonal matmuls with the residual folded into the lag-0
tap, then a pairwise peer exchange and half of the output projection.

Structure (v3):
  - software-pipelined emission: V-projection tiles and two-slot-lookahead
    QK projection chunks are interleaved between attention (pair, q-block)
    slots, so the scalar engine's exp stream starts ~10us in and the PE
    never waits on freshly copied qT/kT.
  - packed attention: the two heads of a pair run their score matmuls
    concurrently as 64x128 row tiles (tile_position T0/T8); causal
    masking is a 0/1 multiply on the exp output (no PE mask prefill, no
    ident stationary reloads); AV matmuls lag one k-tile group so they
    never wait on the exp; softmax denominators ride along as 64 ones
    rows in the [ones | V_h] AV stationary.
  - peer exchange via pairwise ReduceScatter of (own+peer): each core
    writes its conv output slab into both shards, RS returns the pair
    sum, and the SBUF-resident own slab is subtracted back out. Output
    is half an AllGather's, and the own half never re-reads from DRAM.
    W_out row blocks are permuted per-core into own/peer slab order.
  - pair 3 (the tail) exchanges in three RS slices matched to conv
    completion ([0:1024], [1024:1536], [1536:2048]); its conv -> shard
    write -> trigger chain and the final out-proj passes are emitted
    under tc.high_priority() so the tile scheduler keeps them off the
    back of the engine queues.
  - out-projection staged by slab availability: pairs {0,1} + peer 0
    accumulate to bf16 partials during pair 3's attention; tb0/1 finish
    with (peer 1, peer 2, own 3, peer 3) after the first RS slice; tb2/3
    run as full 8-slab [128,1024] psum groups gated only on their slice.
  - conv bias via per-partition tensor_scalar_add (no bias matmul).
"""

import numpy as np
import ml_dtypes

import concourse.bacc as bacc
import concourse.mybir as mybir
import concourse.tile as tile
from concourse.bass_utils import run_bass_kernel_spmd

F32R = mybir.dt.float32r
F32 = mybir.dt.float32
BF16 = mybir.dt.bfloat16

B, T, C, H, D, K = 4, 2048, 1024, 16, 64, 4
HC = H // 2  # heads per core (8)
CC = C // 2  # channels per core (512)
NEG = -30000.0
NCORES = 8
REPLICA_GROUPS = [[0, 1], [2, 3], [4, 5], [6, 7]]
NTB = T // 512  # 512-wide t blocks (4)
NTT = T // 128  # 128-wide t tiles (16)
NCT = C // 128  # 128-wide input-channel tiles (8)
NPAIR = 4  # head pairs per core

_NC_CACHE = {}


def build(debug=False, reps=1, qkv_dt=BF16, packed=True):
    nc = bacc.Bacc(None, num_devices=NCORES)

    xT_d = nc.dram_tensor("xT", [C, T], qkv_dt, kind="ExternalInput")
    wqk_d = nc.dram_tensor("wqk", [C, 1024], qkv_dt, kind="ExternalInput")
    wv_d = nc.dram_tensor("wv", [C, CC], qkv_dt, kind="ExternalInput")
    wout_d = nc.dram_tensor("wout", [C, CC], BF16, kind="ExternalInput")
    if packed:
        mask01_d = nc.dram_tensor("mask01", [128, 512], BF16,
                                  kind="ExternalInput")
        ident_d = masks_d = None
    else:
        ident_d = nc.dram_tensor("ident", [128, 128], BF16,
                                 kind="ExternalInput")
        masks_d = nc.dram_tensor("masks", [128, 512], BF16,
                                 kind="ExternalInput")
        mask01_d = None
    convdiag_d = nc.dram_tensor("convdiag", [128, NPAIR * K, 128], BF16,
                                kind="ExternalInput")
    convbias_d = nc.dram_tensor("convbias", [128, NPAIR], F32,
                                kind="ExternalInput")
    outT_d = nc.dram_tensor("outT", [CC, T], BF16, kind="ExternalOutput")

    with tile.TileContext(nc) as tc:
        with (
            tc.tile_pool(name="consts", bufs=1) as consts,
            tc.tile_pool(name="work", bufs=2) as work,
            tc.tile_pool(name="ps_st", bufs=2, space="PSUM") as ps_st,
            tc.tile_pool(name="ps_att", bufs=1, space="PSUM") as ps_att,
            tc.tile_pool(name="ps_mm", bufs=2, space="PSUM") as ps_mm,
            tc.tile_pool(name="dram", bufs=1, space="DRAM") as dram,
        ):
            for rep in range(reps):
                _emit_body(nc, tc, consts, work, ps_st, ps_att, ps_mm, dram,
                           {"xT_d": xT_d, "wqk_d": wqk_d, "wv_d": wv_d,
                            "wout_d": wout_d, "ident_d": ident_d,
                            "masks_d": masks_d, "mask01_d": mask01_d,
                            "convdiag_d": convdiag_d,
                            "convbias_d": convbias_d, "outT_d": outT_d},
                           rep, packed)

    nc.compile()
    return nc


def _emit_body(nc, tc, consts, work, ps_st, ps_att, ps_mm, dram, env, rep,
               packed=False):
    xT_d = env["xT_d"]
    wqk_d = env["wqk_d"]
    wv_d = env["wv_d"]
    wout_d = env["wout_d"]
    ident_d = env["ident_d"]
    masks_d = env["masks_d"]
    convdiag_d = env["convdiag_d"]
    convbias_d = env["convbias_d"]
    outT_d = env["outT_d"]
    R = f"r{rep}_"

    # ---------- constant loads ----------
    # wqk for pair 0 first so attention slot 0 unblocks early.
    wqk = [
        work.tile([128, NCT, 256], wqk_d.dtype, tag="wqk", bufs=2,
                  name=f"{R}wqk{p}")
        for p in range(NPAIR)
    ]
    nc.sync.dma_start(
        wqk[0][:],
        wqk_d[:, 0:256].rearrange("(n p) m -> p n m", p=128),
    )

    xT = consts.tile([128, NCT, T], xT_d.dtype, tag="xT", name=R + "xT")
    wv = consts.tile([128, NCT, CC], wv_d.dtype, tag="wv", name=R + "wv")
    xT_r = xT_d.rearrange("(n p) m -> p n m", p=128)
    wv_r = wv_d.rearrange("(n p) m -> p n m", p=128)
    # t-block-major load order: the first attention slot only needs the
    # first 512 t-columns of every channel tile
    for ct in range(NCT):
        nc.sync.dma_start(xT[:, ct, 0:512], xT_r[:, ct, 0:512])
        nc.sync.dma_start(wv[:, ct, :], wv_r[:, ct, :])
    for tb in range(1, NTB):
        for ct in range(NCT):
            nc.sync.dma_start(xT[:, ct, 512 * tb: 512 * tb + 512],
                              xT_r[:, ct, 512 * tb: 512 * tb + 512])

    if packed:
        mask01_d = env["mask01_d"]
        mask01 = consts.tile([128, 512], BF16, tag="mask01", name=R + "mask01")
        nc.sync.dma_start(mask01[:], mask01_d[:])
    else:
        ident = consts.tile([128, 128], BF16, tag="ident", name=R + "ident")
        masks = consts.tile([128, 512], BF16, tag="masks", name=R + "masks")
        nc.sync.dma_start(ident[:], ident_d[:])
        nc.sync.dma_start(masks[:], masks_d[:])
    convdiag = consts.tile([128, NPAIR * K, 128], BF16, tag="convdiag",
                           name=R + "convdiag")
    nc.sync.dma_start(convdiag[:], convdiag_d[:])
    convbias = consts.tile([128, NPAIR], F32, tag="convbias",
                           name=R + "convbias")
    nc.sync.dma_start(convbias[:], convbias_d[:])

    wout = consts.tile([128, NCT, CC], BF16, tag="wout", name=R + "wout")
    nc.sync.dma_start(wout[:], wout_d.rearrange("(n p) m -> p n m", p=128))

    # [ones(0:64) | V_h(64:128)] stationary blocks for the AV matmuls
    v_ones = consts.tile([128, NTT, HC, 128], BF16, tag="v_ones",
                         name=R + "v_ones")
    nc.gpsimd.memset(v_ones[:, :, :, 0:64], 1.0)

    # conv output slabs: own half stays in SBUF; peer half arrives via a
    # pairwise ReduceScatter of (own+peer) minus the SBUF-resident own slab.
    y2own = consts.tile([128, NPAIR, T], BF16, tag="y2own", name=R + "y2own")
    y2peer = consts.tile([128, NPAIR, T], BF16, tag="y2peer", name=R + "y2peer")
    # bf16 partial out-proj accumulators
    osbA = consts.tile([128, NTB, 4, 512], BF16, tag="osbA", name=R + "osbA")

    qT = [work.tile([128, T], BF16, tag="qT", bufs=2, name=f"{R}qT{p}")
          for p in range(NPAIR)]
    kT = [work.tile([128, T], BF16, tag="kT", bufs=2, name=f"{R}kT{p}")
          for p in range(NPAIR)]
    yt = [work.tile([128, T], BF16, tag="yt", bufs=2, name=f"{R}yt{p}")
          for p in range(NPAIR)]

    # DRAM staging for the collectives: shard-major RS input [2, 128, T]
    rs_in = [dram.tile([2, 128, T], BF16, tag=f"rsin{p}", name=f"{R}rsin{p}")
             for p in range(3)]
    rs_out = [dram.tile([128, T], BF16, tag=f"rsout{p}", name=f"{R}rsout{p}")
              for p in range(3)]
    # pair 3 exchanged in three slices matched to conv completion times
    RS3_SLICES = ((0, 1024), (1024, 1536), (1536, 2048))
    rs_in3 = [dram.tile([2, 128, hi - lo], BF16, tag=f"rsin3{k}",
                        name=f"{R}rsin3{k}")
              for k, (lo, hi) in enumerate(RS3_SLICES)]
    rs_out3 = [dram.tile([128, hi - lo], BF16, tag=f"rsout3{k}",
                         name=f"{R}rsout3{k}")
               for k, (lo, hi) in enumerate(RS3_SLICES)]

    # ---------- emission helpers ----------
    def emit_v_tt(tt):
        vps = ps_mm.tile([128, 512], F32, tag="mm", name=f"{R}vps{tt}")
        for ct in range(NCT):
            nc.tensor.matmul(
                vps[:],
                xT[:, ct, tt * 128: tt * 128 + 128],
                wv[:, ct, :],
                start=(ct == 0),
                stop=(ct == NCT - 1),
            )
        nc.vector.tensor_copy(v_ones[:, tt, :, 64:128], vps[:])

    def emit_wqk_load(p):
        nc.sync.dma_start(
            wqk[p][:],
            wqk_d[:, 256 * p: 256 * p + 256].rearrange("(n p) m -> p n m",
                                                       p=128),
        )

    def emit_qk_chunk(p, fs, tb):
        # fs 0 -> qT, 1 -> kT; one 512-wide t block
        dst = (qT if fs == 0 else kT)[p]
        ps = ps_mm.tile([128, 512], F32, tag="mm", name=f"{R}qkps{p}_{fs}_{tb}")
        for ct in range(NCT):
            nc.tensor.matmul(
                ps[:],
                wqk[p][:, ct, 128 * fs: 128 * fs + 128],
                xT[:, ct, 512 * tb: 512 * tb + 512],
                start=(ct == 0),
                stop=(ct == NCT - 1),
            )
        nc.vector.tensor_copy(dst[:, 512 * tb: 512 * tb + 512], ps[:])

    def emit_attn_packed(p, qb):
        # both heads' score matmuls run concurrently as 64x128 row tiles
        # (T0/T8); causal masking via 0/1 multiply on the exp output; AV
        # matmuls lag one k-tile group so they never wait on the exp.
        q0 = 512 * qb
        att = [
            ps_att.tile([128, 512], F32, tag=f"att{h}", bufs=1,
                        name=f"{R}att{p}_{qb}_{h}")
            for h in range(2)
        ]
        ngrp = 2 * qb + 2

        def av_for(pend):
            pts, w0s, grp = pend
            for h in range(2):
                for half in range(2):
                    kt = 2 * grp + half
                    w0 = w0s[half]
                    base = 512 * half
                    nc.tensor.matmul(
                        att[h][:, w0:512],
                        v_ones[:, kt, 2 * p + h, :],
                        pts[h][:, base + w0: base + 512],
                        start=(kt == 0),
                        stop=(kt == 4 * qb + 3),
                    )

        pend = None
        for grp in range(ngrp):
            w0s = [max(0, 128 * (2 * grp + half - 4 * qb))
                   for half in range(2)]
            sts = [
                ps_st.tile([128, 1024], F32, tag="st", bufs=2,
                           name=f"{R}st{p}_{qb}_{grp}_{h}")
                for h in range(2)
            ]
            for half in range(2):
                kt = 2 * grp + half
                w0 = w0s[half]
                base = 512 * half
                for h in range(2):
                    hp = 64 * h
                    nc.tensor.matmul(
                        sts[h][:, base + w0: base + 512],
                        kT[p][hp: hp + 64, 128 * kt: 128 * kt + 128],
                        qT[p][hp: hp + 64, q0 + w0: q0 + 512],
                        start=True,
                        stop=True,
                        tile_position=(hp, 0),
                    )
            pts = []
            for h in range(2):
                pt = work.tile([128, 1024], BF16, tag="pt", bufs=8,
                               name=f"{R}pt{p}_{qb}_{grp}_{h}")
                nc.scalar.activation(
                    out=pt[:, w0s[0]: 1024],
                    in_=sts[h][:, w0s[0]: 1024],
                    func=mybir.ActivationFunctionType.Exp,
                    scale=0.125,
                )
                pts.append(pt)
            for half in range(2):
                kt = 2 * grp + half
                i = kt - 4 * qb
                if 0 <= i <= 3:
                    w0 = w0s[half]
                    base = 512 * half
                    for h in range(2):
                        nc.vector.tensor_mul(
                            out=pts[h][:, base + w0: base + w0 + 128],
                            in0=pts[h][:, base + w0: base + w0 + 128],
                            in1=mask01[:, 128 * i: 128 * i + 128],
                        )
            if pend is not None:
                av_for(pend)
            pend = (pts, w0s, grp)
        av_for(pend)
        for h in range(2):
            rec = work.tile([64, 512], F32, tag="rec", bufs=2,
                            name=f"{R}rec{p}_{qb}_{h}")
            nc.vector.reciprocal_approx_fast(rec[:], att[h][0:64, :])
            nc.vector.tensor_mul(
                out=yt[p][64 * h: 64 * h + 64, q0: q0 + 512],
                in0=att[h][64:128, :],
                in1=rec[:],
            )

    def emit_attn_plain(p, qb):
        q0 = 512 * qb
        att = [
            ps_att.tile([128, 512], F32, tag=f"att{h}", bufs=1,
                        name=f"{R}att{p}_{qb}_{h}")
            for h in range(2)
        ]
        ngrp = 2 * qb + 2
        for grp in range(ngrp):
            for h in range(2):
                hp = 64 * h
                st = ps_st.tile([128, 1024], F32, tag="st", bufs=2,
                                name=f"{R}st{p}_{qb}_{grp}_{h}")
                w0s = []
                for half in range(2):
                    kt = 2 * grp + half
                    w0 = max(0, 128 * (kt - 4 * qb))
                    w0s.append(w0)
                    base = 512 * half
                    if w0 > 0 or kt >= 4 * qb:
                        i = kt - 4 * qb
                        nc.tensor.matmul(
                            st[:, base + w0: base + w0 + 128],
                            ident[:],
                            masks[:, 128 * i: 128 * i + 128],
                            start=True,
                            stop=False,
                        )
                        sc_start = False
                    else:
                        sc_start = True
                    nc.tensor.matmul(
                        st[:, base + w0: base + 512],
                        kT[p][hp: hp + 64, 128 * kt: 128 * kt + 128],
                        qT[p][hp: hp + 64, q0 + w0: q0 + 512],
                        start=sc_start,
                        stop=True,
                    )
                pt = work.tile([128, 1024], BF16, tag="pt", bufs=8,
                               name=f"{R}pt{p}_{qb}_{grp}_{h}")
                nc.scalar.activation(
                    out=pt[:, w0s[0]: 1024],
                    in_=st[:, w0s[0]: 1024],
                    func=mybir.ActivationFunctionType.Exp,
                    scale=0.125,
                )
                for half in range(2):
                    kt = 2 * grp + half
                    w0 = w0s[half]
                    base = 512 * half
                    nc.tensor.matmul(
                        att[h][:, w0:512],
                        v_ones[:, kt, 2 * p + h, :],
                        pt[:, base + w0: base + 512],
                        start=(kt == 0),
                        stop=(kt == 4 * qb + 3),
                    )
        for h in range(2):
            rec = work.tile([64, 512], F32, tag="rec", bufs=2,
                            name=f"{R}rec{p}_{qb}_{h}")
            nc.vector.reciprocal_approx_fast(rec[:], att[h][0:64, :])
            nc.vector.tensor_mul(
                out=yt[p][64 * h: 64 * h + 64, q0: q0 + 512],
                in0=att[h][64:128, :],
                in1=rec[:],
            )

    emit_attn = emit_attn_packed if packed else emit_attn_plain

    def emit_conv(p, tb):
        t0 = 512 * tb
        cps = ps_mm.tile([128, 512], F32, tag="mm", name=f"{R}cps{p}_{tb}")
        for lag in range(4):
            j = 3 - lag  # tap index; lag 0 tap has +1 residual folded in
            lo = max(0, lag - t0)
            nc.tensor.matmul(
                cps[:, lo:512],
                convdiag[:, K * p + j, :],
                yt[p][:, t0 + lo - lag: t0 + 512 - lag],
                start=(lag == 0),
                stop=(lag == 3),
            )
        nc.vector.tensor_scalar_add(y2own[:, p, t0: t0 + 512], cps[:],
                                    convbias[:, p: p + 1])
        for shard in range(2):
            if p < 3:
                nc.gpsimd.dma_start(rs_in[p][shard, :, t0: t0 + 512],
                                    y2own[:, p, t0: t0 + 512])
            else:
                k = 0 if tb < 2 else tb - 1
                off = t0 - RS3_SLICES[k][0]
                nc.gpsimd.dma_start(rs_in3[k][shard, :, off: off + 512],
                                    y2own[:, p, t0: t0 + 512])

    def emit_rs(p):
        nc.gpsimd.collective_compute(
            "ReduceScatter",
            mybir.AluOpType.add,
            replica_groups=REPLICA_GROUPS,
            ins=[rs_in[p].opt()],
            outs=[rs_out[p].opt()],
        )

    def emit_rs3(k):
        nc.gpsimd.collective_compute(
            "ReduceScatter",
            mybir.AluOpType.add,
            replica_groups=REPLICA_GROUPS,
            ins=[rs_in3[k].opt()],
            outs=[rs_out3[k].opt()],
        )

    def emit_readback(p):
        rbs = work.tile([128, T], BF16, tag="rbs", bufs=2, name=f"{R}rbs{p}")
        nc.sync.dma_start(rbs[:], rs_out[p][:])
        nc.gpsimd.tensor_sub(y2peer[:, p, :], rbs[:], y2own[:, p, :])

    def emit_readback3(k):
        lo, hi = RS3_SLICES[k]
        rbs = work.tile([128, hi - lo], BF16, tag="rbs3", bufs=2,
                        name=f"{R}rbs3{k}")
        nc.sync.dma_start(rbs[:], rs_out3[k][:])
        nc.gpsimd.tensor_sub(y2peer[:, 3, lo:hi], rbs[:],
                             y2own[:, 3, lo:hi])

    # slabs: s = p (own pair p), s = 4+p (peer pair p); wout rows permuted
    # per-core so block s matches the slab's global channel block.
    def _slab(s, t0):
        if s < 4:
            return y2own[:, s, t0: t0 + 512]
        return y2peer[:, s - 4, t0: t0 + 512]

    def _outproj_pass(tb, ot, slabs, nm):
        t0 = 512 * tb
        ops_ = ps_mm.tile([128, 512], F32, tag="mm", name=f"{R}{nm}{tb}_{ot}")
        for n, s in enumerate(slabs):
            nc.tensor.matmul(
                ops_[:],
                wout[:, s, 128 * ot: 128 * ot + 128],
                _slab(s, t0),
                start=(n == 0),
                stop=(n == len(slabs) - 1),
            )
        return ops_

    def emit_outprojA(tb, ot):
        # own pairs 0-2 + peer pair 0 -> bf16 partial
        ops_ = _outproj_pass(tb, ot, (0, 1, 2, 4), "opsA")
        nc.vector.tensor_copy(osbA[:, tb, ot, :], ops_[:])

    def _outproj_pair(tb, op, slabs, nm):
        # two 128-row output blocks (ot = 2op, 2op+1) in one [128,1024]
        # psum group from the ps_st pool (attention is done by tail time)
        t0 = 512 * tb
        ps = ps_st.tile([128, 1024], F32, tag="st", bufs=2,
                        name=f"{R}{nm}{tb}_{op}")
        for k in range(2):
            ot = 2 * op + k
            for n, s in enumerate(slabs):
                nc.tensor.matmul(
                    ps[:, 512 * k: 512 * k + 512],
                    wout[:, s, 128 * ot: 128 * ot + 128],
                    _slab(s, t0),
                    start=(n == 0),
                    stop=(n == len(slabs) - 1),
                )
        return ps

    def _osb_store(tb, op, osb2):
        dst = outT_d[256 * op: 256 * op + 256,
                     512 * tb: 512 * tb + 512].rearrange(
            "(a p) m -> p a m", p=128)
        nc.gpsimd.dma_start(dst, osb2[:])

    def emit_outprojB(tb, op):
        # peer pairs 1-2, own pair 3, peer pair 3 finish the tb0/1 sum
        ps = _outproj_pair(tb, op, (5, 6, 3, 7), "opsB")
        osb2 = work.tile([128, 2, 512], BF16, tag="osb", bufs=4,
                         name=f"{R}osb{tb}_{op}")
        nc.vector.tensor_add(osb2[:], ps[:],
                             osbA[:, tb, 2 * op: 2 * op + 2, :])
        _osb_store(tb, op, osb2)

    def emit_outproj_full(tb, op):
        # all 8 slabs in one accumulation; used for the last two t-blocks
        ps = _outproj_pair(tb, op, (0, 1, 2, 4, 5, 6, 3, 7), "opsF")
        osb2 = work.tile([128, 2, 512], BF16, tag="osb", bufs=4,
                         name=f"{R}osbF{tb}_{op}")
        nc.vector.tensor_copy(osb2[:], ps[:])
        _osb_store(tb, op, osb2)

    # ---------- schedule ----------
    for tt in range(4):
        emit_v_tt(tt)
    for j in (0, 1):
        emit_qk_chunk(j // 4, 1, j % 4)
        emit_qk_chunk(j // 4, 0, j % 4)

    for i in range(16):
        p, qb = i // 4, i % 4
        emit_attn(p, qb)
        if i < 3:
            for tt in range(4 * (i + 1), 4 * (i + 2)):
                emit_v_tt(tt)
        j = i + 2  # two-slot QK lookahead keeps kT/qT copies off the
        if j <= 15:  # next slot's critical path
            p2, qb2 = j // 4, j % 4
            if qb2 == 0:
                emit_wqk_load(p2)
            emit_qk_chunk(p2, 1, qb2)
            emit_qk_chunk(p2, 0, qb2)
        if p == 3:
            # pair 3's conv -> shard-write -> collective chain is the tail
            # critical path; pull it ahead in every engine's stream
            with tc.high_priority():
                emit_conv(p, qb)
                if qb == 1:
                    emit_rs3(0)
                if qb == 2:
                    emit_rs3(1)
                if qb == 3:
                    emit_rs3(2)
        else:
            emit_conv(p, qb)
            if qb == 3:
                emit_rs(p)
        # readbacks + partial out-proj staged by pair availability
        if i == 7:
            emit_readback(0)
        if i == 11:
            emit_readback(1)
        if i == 13:
            emit_readback(2)
        if p == 3 and qb < 2:
            for ot in range(4):
                emit_outprojA(qb, ot)
    with tc.high_priority():
        emit_readback3(0)
        for tb in (0, 1):
            for op in range(2):
                emit_outprojB(tb, op)
        emit_readback3(1)
        for op in range(2):
            emit_outproj_full(2, op)
        emit_readback3(2)
        for op in range(2):
            emit_outproj_full(3, op)


def _make_masks():
    kp = np.arange(128)[:, None]
    col = np.arange(128)[None, :]
    masks = np.zeros((128, 512), np.float32)
    for i in range(4):
        masks[:, 128 * i: 128 * i + 128] = np.where(kp > col, NEG, 0.0)
    return masks.astype(ml_dtypes.bfloat16)


def _make_mask01():
    kp = np.arange(128)[:, None]
    col = np.arange(128)[None, :]
    m = np.zeros((128, 512), np.float32)
    for i in range(4):
        m[:, 128 * i: 128 * i + 128] = np.where(kp > col, 0.0, 1.0)
    return m.astype(ml_dtypes.bfloat16)


def prepare_in_maps(x, W_qkv, W_out, conv_w, conv_b, qkv_np=ml_dtypes.bfloat16):
    x = np.asarray(x, np.float32)
    W_qkv = np.asarray(W_qkv, np.float32)
    W_out = np.asarray(W_out, np.float32)
    conv_w = np.asarray(conv_w, np.float32).reshape(C, K)
    conv_b = np.asarray(conv_b, np.float32)

    ident = np.eye(128, dtype=np.float32).astype(ml_dtypes.bfloat16)
    masks = _make_masks()
    mask01 = _make_mask01()

    in_maps = []
    for core in range(NCORES):
        b, g = core // 2, core % 2
        xT = np.ascontiguousarray(x[b].T)  # [C, T]
        # wqk: cols [256p:256p+128] = q rows of pair p (.T), then k rows
        wqk = np.empty((C, 1024), np.float32)
        for p in range(NPAIR):
            r0 = 64 * (8 * g + 2 * p)
            wqk[:, 256 * p: 256 * p + 128] = W_qkv[r0: r0 + 128, :].T
            wqk[:, 256 * p + 128: 256 * p + 256] = W_qkv[
                1024 + r0: 1024 + r0 + 128, :
            ].T
        wv = np.ascontiguousarray(W_qkv[2048 + CC * g: 2048 + CC * g + CC, :].T)
        # W_out columns for this core's output slice; row blocks permuted
        # to the slab order: s = own pair p (global block 4g+p) for s<4,
        # s = 4+p -> peer pair p (global block 4(1-g)+p)
        woutT = W_out[CC * g: CC * g + CC, :].T  # [C, CC]
        wout = np.empty_like(woutT)
        for s in range(8):
            src = 4 * g + s if s < 4 else 4 * (1 - g) + (s - 4)
            wout[128 * s: 128 * s + 128, :] = woutT[128 * src: 128 * src + 128, :]
        wout = np.ascontiguousarray(wout).astype(ml_dtypes.bfloat16)
        # conv diag matrices for this core's 4 channel tiles x 4 taps
        convdiag = np.zeros((128, NPAIR * K, 128), np.float32)
        idx = np.arange(128)
        for p in range(NPAIR):
            for j in range(K):
                w = conv_w[CC * g + 128 * p: CC * g + 128 * p + 128, j]
                if j == K - 1:
                    w = w + 1.0  # residual folded into the lag-0 tap
                convdiag[idx, K * p + j, idx] = w
        convbias = np.empty((128, NPAIR), np.float32)
        for p in range(NPAIR):
            convbias[:, p] = conv_b[CC * g + 128 * p: CC * g + 128 * p + 128]
        in_maps.append(
            {
                "xT": xT.astype(qkv_np),
                "wqk": wqk.astype(qkv_np),
                "wv": wv.astype(qkv_np),
                "wout": wout,
                "ident": ident,
                "masks": masks,
                "mask01": mask01,
                "convdiag": convdiag.astype(ml_dtypes.bfloat16),
                "convbias": convbias,
            }
        )
    return in_maps


def assemble_output(results):
    out = np.empty((B, T, C), np.float32)
    for core in range(NCORES):
        b, g = core // 2, core % 2
        outT = np.asarray(results[core]["outT"], np.float32)  # [CC, T]
        out[b, :, CC * g: CC * g + CC] = outT.T
    return out


def kernel(x, W_qkv, W_out, conv_w, conv_b):
    if "nc" not in _NC_CACHE:
        _NC_CACHE["nc"] = build()
    nc = _NC_CACHE["nc"]
    in_maps = prepare_in_maps(x, W_qkv, W_out, conv_w, conv_b)
    res = run_bass_kernel_spmd(nc, in_maps, list(range(NCORES)))
    return assemble_output(res.results)


# revision 54
# speedup vs baseline: 1.0983x; 1.0983x over previous
"""Causal self-attention + depthwise-conv + out-proj fused TRN2 kernel (v3).

Model (B=4, T=2048, C=1024, H=16, D=64, conv K=4):
    qkv = x @ W_qkv.T ; causal softmax attention per head ;
    y2 = attn + causal_depthwise_conv(attn) + conv_b ; out = y2 @ W_out.T

Sharding over 8 NeuronCores: core c -> (batch b = c//2, head-group g = c%2).
Each core computes q/k/v for its 8 heads, bf16 flash-style causal attention
in transposed [d, t] layout (exp softmax without max subtraction), the
depthwise conv as diagonal matmuls with the residual folded into the lag-0
tap, then a pairwise peer exchange and half of the output projection.

Structure (v3):
  - software-pipelined emission: V-projection tiles and two-slot-lookahead
    QK projection chunks are interleaved between attention (pair, q-block)
    slots, so the scalar engine's exp stream starts ~10us in and the PE
    never waits on freshly copied qT/kT.
  - packed attention: the two heads of a pair run their score matmuls
    concurrently as 64x128 row tiles (tile_position T0/T8); causal
    masking is a 0/1 multiply on the exp output (no PE mask prefill, no
    ident stationary reloads); AV matmuls lag one k-tile group so they
    never wait on the exp; softmax denominators ride along as 64 ones
    rows in the [ones | V_h] AV stationary.
  - peer exchange via pairwise ReduceScatter of (own+peer): each core
    writes its conv output slab into both shards, RS returns the pair
    sum, and the SBUF-resident own slab is subtracted back out. Output
    is half an AllGather's, and the own half never re-reads from DRAM.
    W_out row blocks are permuted per-core into own/peer slab order.
  - pair 3 (the tail) exchanges in three RS slices matched to conv
    completion ([0:1024], [1024:1536], [1536:2048]); its conv -> shard
    write -> trigger chain and the final out-proj passes are emitted
    under tc.high_priority() so the tile scheduler keeps them off the
    back of the engine queues.
  - out-projection staged by slab availability: pairs {0,1} + peer 0
    accumulate to bf16 partials during pair 3's attention; tb0/1 finish
    with (peer 1, peer 2, own 3, peer 3) after the first RS slice; tb2/3
    run as full 8-slab [128,1024] psum groups gated only on their slice.
  - conv bias via per-partition tensor_scalar_add (no bias matmul).
"""

import numpy as np
import ml_dtypes

import concourse.bacc as bacc
import concourse.mybir as mybir
import concourse.tile as tile
from concourse.bass_utils import run_bass_kernel_spmd

F32R = mybir.dt.float32r
F32 = mybir.dt.float32
BF16 = mybir.dt.bfloat16

B, T, C, H, D, K = 4, 2048, 1024, 16, 64, 4
HC = H // 2  # heads per core (8)
CC = C // 2  # channels per core (512)
NEG = -30000.0
NCORES = 8
REPLICA_GROUPS = [[0, 1], [2, 3], [4, 5], [6, 7]]
NTB = T // 512  # 512-wide t blocks (4)
NTT = T // 128  # 128-wide t tiles (16)
NCT = C // 128  # 128-wide input-channel tiles (8)
NPAIR = 4  # head pairs per core

_NC_CACHE = {}


def build(debug=False, reps=1, qkv_dt=BF16, packed=True, interleave=False):
    nc = bacc.Bacc(None, num_devices=NCORES)

    xT_d = nc.dram_tensor("xT", [C, T], qkv_dt, kind="ExternalInput")
    wqk_d = nc.dram_tensor("wqk", [C, 1024], qkv_dt, kind="ExternalInput")
    wv_d = nc.dram_tensor("wv", [C, CC], qkv_dt, kind="ExternalInput")
    wout_d = nc.dram_tensor("wout", [C, CC], BF16, kind="ExternalInput")
    if packed:
        mask01_d = nc.dram_tensor("mask01", [128, 512], BF16,
                                  kind="ExternalInput")
        ident_d = masks_d = None
    else:
        ident_d = nc.dram_tensor("ident", [128, 128], BF16,
                                 kind="ExternalInput")
        masks_d = nc.dram_tensor("masks", [128, 512], BF16,
                                 kind="ExternalInput")
        mask01_d = None
    convdiag_d = nc.dram_tensor("convdiag", [128, NPAIR * K, 128], BF16,
                                kind="ExternalInput")
    convbias_d = nc.dram_tensor("convbias", [128, NPAIR], F32,
                                kind="ExternalInput")
    outT_d = nc.dram_tensor("outT", [CC, T], BF16, kind="ExternalOutput")

    with tile.TileContext(nc) as tc:
        with (
            tc.tile_pool(name="consts", bufs=1) as consts,
            tc.tile_pool(name="work", bufs=2) as work,
            tc.tile_pool(name="ps_st", bufs=2, space="PSUM") as ps_st,
            tc.tile_pool(name="ps_att", bufs=1, space="PSUM") as ps_att,
            tc.tile_pool(name="ps_mm", bufs=2, space="PSUM") as ps_mm,
            tc.tile_pool(name="dram", bufs=1, space="DRAM") as dram,
        ):
            for rep in range(reps):
                _emit_body(nc, tc, consts, work, ps_st, ps_att, ps_mm, dram,
                           {"xT_d": xT_d, "wqk_d": wqk_d, "wv_d": wv_d,
                            "wout_d": wout_d, "ident_d": ident_d,
                            "masks_d": masks_d, "mask01_d": mask01_d,
                            "convdiag_d": convdiag_d,
                            "convbias_d": convbias_d, "outT_d": outT_d},
                           rep, packed, interleave)

    nc.compile()
    return nc


def _emit_body(nc, tc, consts, work, ps_st, ps_att, ps_mm, dram, env, rep,
               packed=False, interleave=True):
    xT_d = env["xT_d"]
    wqk_d = env["wqk_d"]
    wv_d = env["wv_d"]
    wout_d = env["wout_d"]
    ident_d = env["ident_d"]
    masks_d = env["masks_d"]
    convdiag_d = env["convdiag_d"]
    convbias_d = env["convbias_d"]
    outT_d = env["outT_d"]
    R = f"r{rep}_"

    # ---------- constant loads ----------
    # wqk for pair 0 first so attention slot 0 unblocks early.
    wqk = [
        work.tile([128, NCT, 256], wqk_d.dtype, tag="wqk", bufs=2,
                  name=f"{R}wqk{p}")
        for p in range(NPAIR)
    ]
    nc.sync.dma_start(
        wqk[0][:],
        wqk_d[:, 0:256].rearrange("(n p) m -> p n m", p=128),
    )

    xT = consts.tile([128, NCT, T], xT_d.dtype, tag="xT", name=R + "xT")
    wv = consts.tile([128, NCT, CC], wv_d.dtype, tag="wv", name=R + "wv")
    xT_r = xT_d.rearrange("(n p) m -> p n m", p=128)
    wv_r = wv_d.rearrange("(n p) m -> p n m", p=128)
    # t-block-major load order: the first attention slot only needs the
    # first 512 t-columns of every channel tile
    for ct in range(NCT):
        nc.sync.dma_start(xT[:, ct, 0:512], xT_r[:, ct, 0:512])
        nc.sync.dma_start(wv[:, ct, :], wv_r[:, ct, :])
    for tb in range(1, NTB):
        for ct in range(NCT):
            nc.sync.dma_start(xT[:, ct, 512 * tb: 512 * tb + 512],
                              xT_r[:, ct, 512 * tb: 512 * tb + 512])

    if packed:
        mask01_d = env["mask01_d"]
        mask01 = consts.tile([128, 512], BF16, tag="mask01", name=R + "mask01")
        nc.sync.dma_start(mask01[:], mask01_d[:])
    else:
        ident = consts.tile([128, 128], BF16, tag="ident", name=R + "ident")
        masks = consts.tile([128, 512], BF16, tag="masks", name=R + "masks")
        nc.sync.dma_start(ident[:], ident_d[:])
        nc.sync.dma_start(masks[:], masks_d[:])
    convdiag = consts.tile([128, NPAIR * K, 128], BF16, tag="convdiag",
                           name=R + "convdiag")
    nc.sync.dma_start(convdiag[:], convdiag_d[:])
    convbias = consts.tile([128, NPAIR], F32, tag="convbias",
                           name=R + "convbias")
    nc.sync.dma_start(convbias[:], convbias_d[:])

    wout = consts.tile([128, NCT, CC], BF16, tag="wout", name=R + "wout")
    nc.sync.dma_start(wout[:], wout_d.rearrange("(n p) m -> p n m", p=128))

    # [ones(0:64) | V_h(64:128)] stationary blocks for the AV matmuls
    v_ones = consts.tile([128, NTT, HC, 128], BF16, tag="v_ones",
                         name=R + "v_ones")
    nc.gpsimd.memset(v_ones[:, :, :, 0:64], 1.0)

    # conv output slabs: own half stays in SBUF; peer half arrives via a
    # pairwise ReduceScatter of (own+peer) minus the SBUF-resident own slab.
    y2own = consts.tile([128, NPAIR, T], BF16, tag="y2own", name=R + "y2own")
    y2peer = consts.tile([128, NPAIR, T], BF16, tag="y2peer", name=R + "y2peer")
    # bf16 partial out-proj accumulators
    osbA = consts.tile([128, NTB, 4, 512], BF16, tag="osbA", name=R + "osbA")

    qT = [work.tile([128, T], BF16, tag="qT", bufs=2, name=f"{R}qT{p}")
          for p in range(NPAIR)]
    kT = [work.tile([128, T], BF16, tag="kT", bufs=2, name=f"{R}kT{p}")
          for p in range(NPAIR)]
    yt = [work.tile([128, T], BF16, tag="yt", bufs=2, name=f"{R}yt{p}")
          for p in range(NPAIR)]

    # DRAM staging for the collectives: shard-major RS input [2, 128, T]
    rs_in = [dram.tile([2, 128, T], BF16, tag=f"rsin{p}", name=f"{R}rsin{p}")
             for p in range(3)]
    rs_out = [dram.tile([128, T], BF16, tag=f"rsout{p}", name=f"{R}rsout{p}")
              for p in range(3)]
    # pair 3 exchanged in three slices matched to conv completion times
    RS3_SLICES = ((0, 1024), (1024, 1536), (1536, 2048))
    rs_in3 = [dram.tile([2, 128, hi - lo], BF16, tag=f"rsin3{k}",
                        name=f"{R}rsin3{k}")
              for k, (lo, hi) in enumerate(RS3_SLICES)]
    rs_out3 = [dram.tile([128, hi - lo], BF16, tag=f"rsout3{k}",
                         name=f"{R}rsout3{k}")
               for k, (lo, hi) in enumerate(RS3_SLICES)]

    # ---------- emission helpers ----------
    def emit_v_tt(tt):
        vps = ps_mm.tile([128, 512], F32, tag="mm", name=f"{R}vps{tt}")
        for ct in range(NCT):
            nc.tensor.matmul(
                vps[:],
                xT[:, ct, tt * 128: tt * 128 + 128],
                wv[:, ct, :],
                start=(ct == 0),
                stop=(ct == NCT - 1),
            )
        nc.vector.tensor_copy(v_ones[:, tt, :, 64:128], vps[:])

    def emit_wqk_load(p):
        nc.sync.dma_start(
            wqk[p][:],
            wqk_d[:, 256 * p: 256 * p + 256].rearrange("(n p) m -> p n m",
                                                       p=128),
        )

    def emit_qk_chunk(p, fs, tb):
        # fs 0 -> qT, 1 -> kT; one 512-wide t block
        dst = (qT if fs == 0 else kT)[p]
        ps = ps_mm.tile([128, 512], F32, tag="mm", name=f"{R}qkps{p}_{fs}_{tb}")
        for ct in range(NCT):
            nc.tensor.matmul(
                ps[:],
                wqk[p][:, ct, 128 * fs: 128 * fs + 128],
                xT[:, ct, 512 * tb: 512 * tb + 512],
                start=(ct == 0),
                stop=(ct == NCT - 1),
            )
        nc.vector.tensor_copy(dst[:, 512 * tb: 512 * tb + 512], ps[:])

    def emit_attn_packed(p, qb, fillers=()):
        # both heads' score matmuls run concurrently as 64x128 row tiles
        # (T0/T8); causal masking via 0/1 multiply on the exp output; AV
        # matmuls lag one k-tile group so they never wait on the exp.
        # `fillers`: independent PE work (V/QK projection groups) spread
        # between k-tile groups — the attention stream is exp-rate-gated,
        # and the in-order PE queue would otherwise idle ~1us per group.
        fillers = list(fillers)
        q0 = 512 * qb
        att = [
            ps_att.tile([128, 512], F32, tag=f"att{h}", bufs=1,
                        name=f"{R}att{p}_{qb}_{h}")
            for h in range(2)
        ]
        ngrp = 2 * qb + 2

        def av_for(pend):
            pts, w0s, grp = pend
            for h in range(2):
                for half in range(2):
                    kt = 2 * grp + half
                    w0 = w0s[half]
                    base = 512 * half
                    nc.tensor.matmul(
                        att[h][:, w0:512],
                        v_ones[:, kt, 2 * p + h, :],
                        pts[h][:, base + w0: base + 512],
                        start=(kt == 0),
                        stop=(kt == 4 * qb + 3),
                    )

        pend = None
        for grp in range(ngrp):
            w0s = [max(0, 128 * (2 * grp + half - 4 * qb))
                   for half in range(2)]
            sts = [
                ps_st.tile([128, 1024], F32, tag="st", bufs=2,
                           name=f"{R}st{p}_{qb}_{grp}_{h}")
                for h in range(2)
            ]
            for half in range(2):
                kt = 2 * grp + half
                w0 = w0s[half]
                base = 512 * half
                for h in range(2):
                    hp = 64 * h
                    nc.tensor.matmul(
                        sts[h][:, base + w0: base + 512],
                        kT[p][hp: hp + 64, 128 * kt: 128 * kt + 128],
                        qT[p][hp: hp + 64, q0 + w0: q0 + 512],
                        start=True,
                        stop=True,
                        tile_position=(hp, 0),
                    )
            pts = []
            for h in range(2):
                pt = work.tile([128, 1024], BF16, tag="pt", bufs=8,
                               name=f"{R}pt{p}_{qb}_{grp}_{h}")
                nc.scalar.activation(
                    out=pt[:, w0s[0]: 1024],
                    in_=sts[h][:, w0s[0]: 1024],
                    func=mybir.ActivationFunctionType.Exp,
                    scale=0.125,
                )
                pts.append(pt)
            for half in range(2):
                kt = 2 * grp + half
                i = kt - 4 * qb
                if 0 <= i <= 3:
                    w0 = w0s[half]
                    base = 512 * half
                    for h in range(2):
                        nc.vector.tensor_mul(
                            out=pts[h][:, base + w0: base + w0 + 128],
                            in0=pts[h][:, base + w0: base + w0 + 128],
                            in1=mask01[:, 128 * i: 128 * i + 128],
                        )
            if pend is not None:
                av_for(pend)
            pend = (pts, w0s, grp)
            if fillers:
                fillers.pop(0)()
        av_for(pend)
        for h in range(2):
            rec = work.tile([64, 512], F32, tag="rec", bufs=2,
                            name=f"{R}rec{p}_{qb}_{h}")
            nc.vector.reciprocal_approx_fast(rec[:], att[h][0:64, :])
            nc.vector.tensor_mul(
                out=yt[p][64 * h: 64 * h + 64, q0: q0 + 512],
                in0=att[h][64:128, :],
                in1=rec[:],
            )
        for f in fillers:
            f()

    def emit_attn_plain(p, qb, fillers=()):
        q0 = 512 * qb
        att = [
            ps_att.tile([128, 512], F32, tag=f"att{h}", bufs=1,
                        name=f"{R}att{p}_{qb}_{h}")
            for h in range(2)
        ]
        ngrp = 2 * qb + 2
        for grp in range(ngrp):
            for h in range(2):
                hp = 64 * h
                st = ps_st.tile([128, 1024], F32, tag="st", bufs=2,
                                name=f"{R}st{p}_{qb}_{grp}_{h}")
                w0s = []
                for half in range(2):
                    kt = 2 * grp + half
                    w0 = max(0, 128 * (kt - 4 * qb))
                    w0s.append(w0)
                    base = 512 * half
                    if w0 > 0 or kt >= 4 * qb:
                        i = kt - 4 * qb
                        nc.tensor.matmul(
                            st[:, base + w0: base + w0 + 128],
                            ident[:],
                            masks[:, 128 * i: 128 * i + 128],
                            start=True,
                            stop=False,
                        )
                        sc_start = False
                    else:
                        sc_start = True
                    nc.tensor.matmul(
                        st[:, base + w0: base + 512],
                        kT[p][hp: hp + 64, 128 * kt: 128 * kt + 128],
                        qT[p][hp: hp + 64, q0 + w0: q0 + 512],
                        start=sc_start,
                        stop=True,
                    )
                pt = work.tile([128, 1024], BF16, tag="pt", bufs=8,
                               name=f"{R}pt{p}_{qb}_{grp}_{h}")
                nc.scalar.activation(
                    out=pt[:, w0s[0]: 1024],
                    in_=st[:, w0s[0]: 1024],
                    func=mybir.ActivationFunctionType.Exp,
                    scale=0.125,
                )
                for half in range(2):
                    kt = 2 * grp + half
                    w0 = w0s[half]
                    base = 512 * half
                    nc.tensor.matmul(
                        att[h][:, w0:512],
                        v_ones[:, kt, 2 * p + h, :],
                        pt[:, base + w0: base + 512],
                        start=(kt == 0),
                        stop=(kt == 4 * qb + 3),
                    )
        for h in range(2):
            rec = work.tile([64, 512], F32, tag="rec", bufs=2,
                            name=f"{R}rec{p}_{qb}_{h}")
            nc.vector.reciprocal_approx_fast(rec[:], att[h][0:64, :])
            nc.vector.tensor_mul(
                out=yt[p][64 * h: 64 * h + 64, q0: q0 + 512],
                in0=att[h][64:128, :],
                in1=rec[:],
            )
        for f in fillers:
            f()

    emit_attn = emit_attn_packed if packed else emit_attn_plain

    def emit_conv(p, tb):
        t0 = 512 * tb
        cps = ps_mm.tile([128, 512], F32, tag="mm", name=f"{R}cps{p}_{tb}")
        for lag in range(4):
            j = 3 - lag  # tap index; lag 0 tap has +1 residual folded in
            lo = max(0, lag - t0)
            nc.tensor.matmul(
                cps[:, lo:512],
                convdiag[:, K * p + j, :],
                yt[p][:, t0 + lo - lag: t0 + 512 - lag],
                start=(lag == 0),
                stop=(lag == 3),
            )
        nc.vector.tensor_scalar_add(y2own[:, p, t0: t0 + 512], cps[:],
                                    convbias[:, p: p + 1])
        for shard in range(2):
            if p < 3:
                nc.gpsimd.dma_start(rs_in[p][shard, :, t0: t0 + 512],
                                    y2own[:, p, t0: t0 + 512])
            else:
                k = 0 if tb < 2 else tb - 1
                off = t0 - RS3_SLICES[k][0]
                nc.gpsimd.dma_start(rs_in3[k][shard, :, off: off + 512],
                                    y2own[:, p, t0: t0 + 512])

    def emit_rs(p):
        nc.gpsimd.collective_compute(
            "ReduceScatter",
            mybir.AluOpType.add,
            replica_groups=REPLICA_GROUPS,
            ins=[rs_in[p].opt()],
            outs=[rs_out[p].opt()],
        )

    def emit_rs3(k):
        nc.gpsimd.collective_compute(
            "ReduceScatter",
            mybir.AluOpType.add,
            replica_groups=REPLICA_GROUPS,
            ins=[rs_in3[k].opt()],
            outs=[rs_out3[k].opt()],
        )

    def emit_readback(p):
        rbs = work.tile([128, T], BF16, tag="rbs", bufs=2, name=f"{R}rbs{p}")
        nc.sync.dma_start(rbs[:], rs_out[p][:])
        nc.gpsimd.tensor_sub(y2peer[:, p, :], rbs[:], y2own[:, p, :])

    def emit_readback3(k):
        lo, hi = RS3_SLICES[k]
        rbs = work.tile([128, hi - lo], BF16, tag="rbs3", bufs=2,
                        name=f"{R}rbs3{k}")
        nc.sync.dma_start(rbs[:], rs_out3[k][:])
        nc.gpsimd.tensor_sub(y2peer[:, 3, lo:hi], rbs[:],
                             y2own[:, 3, lo:hi])

    # slabs: s = p (own pair p), s = 4+p (peer pair p); wout rows permuted
    # per-core so block s matches the slab's global channel block.
    def _slab(s, t0):
        if s < 4:
            return y2own[:, s, t0: t0 + 512]
        return y2peer[:, s - 4, t0: t0 + 512]

    def _outproj_pass(tb, ot, slabs, nm):
        t0 = 512 * tb
        ops_ = ps_mm.tile([128, 512], F32, tag="mm", name=f"{R}{nm}{tb}_{ot}")
        for n, s in enumerate(slabs):
            nc.tensor.matmul(
                ops_[:],
                wout[:, s, 128 * ot: 128 * ot + 128],
                _slab(s, t0),
                start=(n == 0),
                stop=(n == len(slabs) - 1),
            )
        return ops_

    def emit_outprojA(tb, ot):
        # own pairs 0-2 + peer pair 0 -> bf16 partial
        ops_ = _outproj_pass(tb, ot, (0, 1, 2, 4), "opsA")
        nc.vector.tensor_copy(osbA[:, tb, ot, :], ops_[:])

    def _outproj_pair(tb, op, slabs, nm):
        # two 128-row output blocks (ot = 2op, 2op+1) in one [128,1024]
        # psum group from the ps_st pool (attention is done by tail time)
        t0 = 512 * tb
        ps = ps_st.tile([128, 1024], F32, tag="st", bufs=2,
                        name=f"{R}{nm}{tb}_{op}")
        for k in range(2):
            ot = 2 * op + k
            for n, s in enumerate(slabs):
                nc.tensor.matmul(
                    ps[:, 512 * k: 512 * k + 512],
                    wout[:, s, 128 * ot: 128 * ot + 128],
                    _slab(s, t0),
                    start=(n == 0),
                    stop=(n == len(slabs) - 1),
                )
        return ps

    def _osb_store(tb, op, osb2):
        dst = outT_d[256 * op: 256 * op + 256,
                     512 * tb: 512 * tb + 512].rearrange(
            "(a p) m -> p a m", p=128)
        nc.gpsimd.dma_start(dst, osb2[:])

    def emit_outprojB(tb, op):
        # peer pairs 1-2, own pair 3, peer pair 3 finish the tb0/1 sum
        ps = _outproj_pair(tb, op, (5, 6, 3, 7), "opsB")
        osb2 = work.tile([128, 2, 512], BF16, tag="osb", bufs=4,
                         name=f"{R}osb{tb}_{op}")
        nc.vector.tensor_add(osb2[:], ps[:],
                             osbA[:, tb, 2 * op: 2 * op + 2, :])
        _osb_store(tb, op, osb2)

    def emit_outproj_full(tb, op):
        # all 8 slabs in one accumulation; used for the last two t-blocks
        ps = _outproj_pair(tb, op, (0, 1, 2, 4, 5, 6, 3, 7), "opsF")
        osb2 = work.tile([128, 2, 512], BF16, tag="osb", bufs=4,
                         name=f"{R}osbF{tb}_{op}")
        nc.vector.tensor_copy(osb2[:], ps[:])
        _osb_store(tb, op, osb2)

    # ---------- schedule ----------
    for tt in range(4):
        emit_v_tt(tt)
    for j in (0, 1):
        emit_qk_chunk(j // 4, 1, j % 4)
        emit_qk_chunk(j // 4, 0, j % 4)

    for i in range(16):
        p, qb = i // 4, i % 4
        fillers = []
        if i < 3:
            for tt in range(4 * (i + 1), 4 * (i + 2)):
                fillers.append(lambda tt=tt: emit_v_tt(tt))
        j = i + 2  # two-slot QK lookahead keeps kT/qT copies off the
        if j <= 15:  # next slot's critical path
            p2, qb2 = j // 4, j % 4
            if qb2 == 0:
                emit_wqk_load(p2)
            fillers.append(lambda a=p2, b=qb2: emit_qk_chunk(a, 1, b))
            fillers.append(lambda a=p2, b=qb2: emit_qk_chunk(a, 0, b))
        opA = []
        if p == 3 and qb < 2:
            for ot in range(4):
                opA.append(lambda a=qb, b=ot: emit_outprojA(a, b))
        if interleave:
            emit_attn(p, qb, fillers + opA)
            opA = []
        else:
            emit_attn(p, qb)
            for f in fillers:
                f()
        if p == 3:
            # pair 3's conv -> shard-write -> collective chain is the tail
            # critical path; pull it ahead in every engine's stream
            with tc.high_priority():
                emit_conv(p, qb)
                if qb == 1:
                    emit_rs3(0)
                if qb == 2:
                    emit_rs3(1)
                if qb == 3:
                    emit_rs3(2)
        else:
            emit_conv(p, qb)
            if qb == 3:
                emit_rs(p)
        # readbacks + partial out-proj staged by pair availability
        if i == 7:
            emit_readback(0)
        if i == 11:
            emit_readback(1)
        if i == 13:
            emit_readback(2)
        for f in opA:
            f()
    with tc.high_priority():
        emit_readback3(0)
        for tb in (0, 1):
            for op in range(2):
                emit_outprojB(tb, op)
        emit_readback3(1)
        for op in range(2):
            emit_outproj_full(2, op)
        emit_readback3(2)
        for op in range(2):
            emit_outproj_full(3, op)


def _make_masks():
    kp = np.arange(128)[:, None]
    col = np.arange(128)[None, :]
    masks = np.zeros((128, 512), np.float32)
    for i in range(4):
        masks[:, 128 * i: 128 * i + 128] = np.where(kp > col, NEG, 0.0)
    return masks.astype(ml_dtypes.bfloat16)


def _make_mask01():
    kp = np.arange(128)[:, None]
    col = np.arange(128)[None, :]
    m = np.zeros((128, 512), np.float32)
    for i in range(4):
        m[:, 128 * i: 128 * i + 128] = np.where(kp > col, 0.0, 1.0)
    return m.astype(ml_dtypes.bfloat16)


def prepare_in_maps(x, W_qkv, W_out, conv_w, conv_b, qkv_np=ml_dtypes.bfloat16):
    x = np.asarray(x, np.float32)
    W_qkv = np.asarray(W_qkv, np.float32)
    W_out = np.asarray(W_out, np.float32)
    conv_w = np.asarray(conv_w, np.float32).reshape(C, K)
    conv_b = np.asarray(conv_b, np.float32)

    ident = np.eye(128, dtype=np.float32).astype(ml_dtypes.bfloat16)
    masks = _make_masks()
    mask01 = _make_mask01()

    in_maps = []
    for core in range(NCORES):
        b, g = core // 2, core % 2
        xT = np.ascontiguousarray(x[b].T)  # [C, T]
        # wqk: cols [256p:256p+128] = q rows of pair p (.T), then k rows
        wqk = np.empty((C, 1024), np.float32)
        for p in range(NPAIR):
            r0 = 64 * (8 * g + 2 * p)
            wqk[:, 256 * p: 256 * p + 128] = W_qkv[r0: r0 + 128, :].T
            wqk[:, 256 * p + 128: 256 * p + 256] = W_qkv[
                1024 + r0: 1024 + r0 + 128, :
            ].T
        wv = np.ascontiguousarray(W_qkv[2048 + CC * g: 2048 + CC * g + CC, :].T)
        # W_out columns for this core's output slice; row blocks permuted
        # to the slab order: s = own pair p (global block 4g+p) for s<4,
        # s = 4+p -> peer pair p (global block 4(1-g)+p)
        woutT = W_out[CC * g: CC * g + CC, :].T  # [C, CC]
        wout = np.empty_like(woutT)
        for s in range(8):
            src = 4 * g + s if s < 4 else 4 * (1 - g) + (s - 4)
            wout[128 * s: 128 * s + 128, :] = woutT[128 * src: 128 * src + 128, :]
        wout = np.ascontiguousarray(wout).astype(ml_dtypes.bfloat16)
        # conv diag matrices for this core's 4 channel tiles x 4 taps
        convdiag = np.zeros((128, NPAIR * K, 128), np.float32)
        idx = np.arange(128)
        for p in range(NPAIR):
            for j in range(K):
                w = conv_w[CC * g + 128 * p: CC * g + 128 * p + 128, j]
                if j == K - 1:
                    w = w + 1.0  # residual folded into the lag-0 tap
                convdiag[idx, K * p + j, idx] = w
        convbias = np.empty((128, NPAIR), np.float32)
        for p in range(NPAIR):
            convbias[:, p] = conv_b[CC * g + 128 * p: CC * g + 128 * p + 128]
        in_maps.append(
            {
                "xT": xT.astype(qkv_np),
                "wqk": wqk.astype(qkv_np),
                "wv": wv.astype(qkv_np),
                "wout": wout,
                "ident": ident,
                "masks": masks,
                "mask01": mask01,
                "convdiag": convdiag.astype(ml_dtypes.bfloat16),
                "convbias": convbias,
            }
        )
    return in_maps


def assemble_output(results):
    out = np.empty((B, T, C), np.float32)
    for core in range(NCORES):
        b, g = core // 2, core % 2
        outT = np.asarray(results[core]["outT"], np.float32)  # [CC, T]
        out[b, :, CC * g: CC * g + CC] = outT.T
    return out


def kernel(x, W_qkv, W_out, conv_w, conv_b):
    if "nc" not in _NC_CACHE:
        _NC_CACHE["nc"] = build()
    nc = _NC_CACHE["nc"]
    in_maps = prepare_in_maps(x, W_qkv, W_out, conv_w, conv_b)
    res = run_bass_kernel_spmd(nc, in_maps, list(range(NCORES)))
    return assemble_output(res.results)
